# revision 34
# baseline (speedup 1.0000x reference)
"""Trainium2 Bass kernel for the CPC (wav2vec-style contrastive) module.

Strategy (data-parallel over batch, 2 batches per core on 8 cores):
  1. proj[d, s, t] = sum_c W[c, d, s] * x[c, t]    (PE matmuls, fp16 in / fp32 acc)
  2. Gather target columns of y per (timestep u, copy n) with dma_gather
     (copy 0 = positive column u itself, copies 1..10 = sampled negatives,
      copy 11 = dummy padding so 12 copies align nicely with 128).
  3. Contrastive logits via block-diagonal matmuls:
        out[(u, n), (i, u')] = sum_d G[d, (u, n)] * proj[d, i, u' - 1 - i]
     Only the u == u' entries are wanted; host extracts that diagonal.
Everything heavy (both einsums and the negative gather) runs on-device.
"""

import numpy as np

import concourse.bass as bass
import concourse.mybir as mybir
import concourse.tile as tile
from concourse import bacc
from concourse.bass_utils import run_bass_kernel_spmd

# Problem constants (hardcoded per contest rules).
B, C, T = 16, 768, 1024
S = 12              # prediction steps
NNEG = 10
COPIES = NNEG + 1   # positive + negatives
CP = 12             # padded copies per timestep (extra dummy)
OFFSET = 1
NCORES = 8
NB = B // NCORES    # batches per core
CH = C // 128       # channel chunks
NQM = 4             # PE-quarters (matmul N granularity)
TQM = T // NQM      # 256
TE = 128            # eighth width (proj buffer granularity)
HIST = 13           # history columns carried between eighths
PW_E = TE + HIST    # 141 = per-(chunk, step) block width in a proj buffer
PQF_E = 6 * 12 * PW_E  # free elems per partition in a proj buffer
GT_U = 32           # timesteps per gather tile
NGT = T // GT_U     # gather tiles per batch
NI = GT_U * CP      # gather indices per tile (must be % 128)
ST_U = 8            # timesteps per contrastive subtile
NSS = GT_U // ST_U  # subtiles per gather tile
NPS = 4             # subtiles per contrastive psum tile / praw row
M_C = ST_U * CP     # 96 psum partitions per contrastive subtile
N_C = S * ST_U      # 96 moving columns per contrastive subtile
PRAW_G = T // (NPS * ST_U)  # 32 praw row-groups per batch

F16 = mybir.dt.float16
F32 = mybir.dt.float32
I16 = mybir.dt.int16

import os as _os
N_SWDGE_Q = int(_os.environ.get("KERN_SWDGE_Q", "1"))
SINGLE_PACKET = bool(int(_os.environ.get("KERN_SINGLE_PACKET", "1")))

_CACHE = {}


def _build(repeat=1, do_proj=True, do_gather=True, do_contr=True,
           do_pevict=True, do_store=True):
    nc = bacc.Bacc(None, target_bir_lowering=False,
                   num_swdge_queues=N_SWDGE_Q)
    x_d = nc.dram_tensor("x2", [NB, C, T], F16, kind="ExternalInput")
    w_d = nc.dram_tensor("w", [S, C, C], F16, kind="ExternalInput")
    yt_d = nc.dram_tensor("yt2", [NB, T, C], F16, kind="ExternalInput")
    idx_d = nc.dram_tensor("idx2", [NB, NGT, 128, NI // 16], I16,
                           kind="ExternalInput")
    praw_d = nc.dram_tensor("praw", [NB, PRAW_G, M_C, NPS * N_C], F16,
                            kind="ExternalOutput")

    with tile.TileContext(nc) as tc:
        with (
            tc.tile_pool(name="wpool", bufs=1) as wpool,
            tc.tile_pool(name="xpool", bufs=1) as xpool,
            tc.tile_pool(name="projpool", bufs=3) as projpool,
            tc.tile_pool(name="gpool", bufs=5) as gpool,
            tc.tile_pool(name="idxpool", bufs=1) as idxpool,
            tc.tile_pool(name="stpool", bufs=2) as stpool,
            tc.tile_pool(name="ppsum", bufs=4, space="PSUM") as ppsum,
            tc.tile_pool(name="cpsum", bufs=4, space="PSUM") as cpsum,
        ):
            # Weights resident for the whole kernel: [p, s, chunk, d].
            # Split the load per step so the first matmuls start early.
            w_sb = wpool.tile([128, S, CH, C], F16)
            for s in range(S):
                nc.sync.dma_start(
                    w_sb[:, s], w_d[s].rearrange("(a p) d -> p a d", p=128))

            def body(_i=None):
                _core_body(nc, tc, x_d, yt_d, idx_d, praw_d, w_sb,
                           xpool, projpool, gpool, idxpool, stpool,
                           ppsum, cpsum, do_proj, do_gather, do_contr,
                           do_pevict, do_store)

            if repeat == 1:
                body()
            else:
                with tc.For_i(0, repeat, 1):
                    body()

    nc.compile()
    return nc


def _core_body(nc, tc, x_d, yt_d, idx_d, praw_d, w_sb,
               xpool, projpool, gpool, idxpool, stpool, ppsum, cpsum,
               do_proj=True, do_gather=True, do_contr=True,
               do_pevict=True, do_store=True):
    for b in range(NB):
        x_sb = xpool.tile([128, CH, T], F16)
        nc.sync.dma_start(x_sb[:],
                          x_d[b].rearrange("(a p) t -> p a t", p=128))
        idx_sb = idxpool.tile([128, NGT, NI // 16], I16)
        nc.sync.dma_start(idx_sb[:],
                          idx_d[b].rearrange("g p c -> p g c"))

        def hist_ap(tile_ap, base_col):
            return bass.AP(tile_ap.tensor, tile_ap.offset + base_col,
                           [[PQF_E, 128], [S * PW_E, CH], [PW_E, S],
                            [1, HIST]])

        prev = None  # previous eighth buffer's AP (for history carry)
        for qm in range(NQM):
            # two eighth-sized proj buffers for this PE-quarter
            pq0 = projpool.tile([128, CH, S, PW_E], F16, tag="pq")
            pq1 = projpool.tile([128, CH, S, PW_E], F16, tag="pq")
            a0, a1 = pq0[:], pq1[:]
            if qm == 0:
                nc.any.memset(hist_ap(a0, 0), 0.0)
            else:
                nc.vector.tensor_copy(hist_ap(a0, 0), hist_ap(prev, TE))

            # ---- projection for this PE-quarter (two eighths) ----
            for m in range(CH if do_proj else 0):
                for sp in range(S // 2):
                    ps = ppsum.tile([128, 2 * TQM], F32)
                    for sh in range(2):
                        s = 2 * sp + sh
                        for k in range(CH):
                            nc.tensor.matmul(
                                ps[:, TQM * sh:TQM * (sh + 1)],
                                w_sb[:, s, k, 128 * m:128 * (m + 1)],
                                x_sb[:, k, TQM * qm:TQM * (qm + 1)],
                                start=(k == 0), stop=(k == CH - 1),
                            )
                    if do_pevict:
                        psv = ps[:].rearrange("p (a j t) -> p a j t",
                                              a=2, j=2)
                        for j, aj in ((0, a0), (1, a1)):
                            dst = bass.AP(
                                aj.tensor,
                                aj.offset + m * (S * PW_E)
                                + (2 * sp) * PW_E + HIST,
                                [[PQF_E, 128], [PW_E, 2], [1, TE]])
                            nc.vector.tensor_copy(dst, psv[:, :, j, :])

            # history carry into the second eighth
            nc.vector.tensor_copy(hist_ap(a1, 0), hist_ap(a0, TE))

            # ---- contrastive for the two eighths ----
            for j, aj in ((0, a0), (1, a1)):
                e = 2 * qm + j
                for g2 in range(TE // GT_U):  # gather tiles per eighth
                    gt = (TE // GT_U) * e + g2
                    G = gpool.tile([128, CH, NI], F16)
                    if do_gather:
                        nc.gpsimd.dma_gather(
                            G[:], yt_d[b], idx_sb[:, gt, :],
                            NI, NI, C, transpose=True,
                            single_packet=SINGLE_PACKET,
                            queue_num=gt % N_SWDGE_Q)
                    elif do_contr:
                        nc.vector.memset(G[:], 0.0)
                    for h in range(NSS // NPS if do_contr else 0):
                        cps = cpsum.tile([128, NPS * N_C], F32)
                        for ssl in range(NPS):
                            ss = h * NPS + ssl
                            u_e = g2 * GT_U + ss * ST_U  # offs in eighth
                            for k in range(CH):
                                rhs = bass.AP(
                                    aj.tensor,
                                    aj.offset + k * (S * PW_E)
                                    + (HIST - 1) + u_e,
                                    [[PQF_E, 128], [PW_E - 1, S],
                                     [1, ST_U]])
                                nc.tensor.matmul(
                                    cps[0:M_C, N_C * ssl:N_C * (ssl + 1)],
                                    G[:, k, M_C * ss:M_C * (ss + 1)],
                                    rhs,
                                    start=(k == 0), stop=(k == CH - 1),
                                )
                        if do_store:
                            stg = stpool.tile([M_C, NPS * N_C], F16)
                            nc.vector.tensor_copy(stg[:], cps[0:M_C, :])
                            nc.sync.dma_start(
                                praw_d[b, gt * (NSS // NPS) + h], stg[:])
            prev = a1


def _get_nc():
    if "nc" not in _CACHE:
        _CACHE["nc"] = _build()
    return _CACHE["nc"]


def _prep_inputs(x, y, W, neg_idxs):
    x16 = x.astype(np.float16)                                     # [B, C, T]
    w16 = np.ascontiguousarray(W.transpose(2, 0, 1).astype(np.float16))  # [S,C,C]
    yt16 = np.ascontiguousarray(np.swapaxes(y, 1, 2).astype(np.float16))  # [B,T,C]

    nl = (neg_idxs.astype(np.int64)
          - (np.arange(B, dtype=np.int64)[:, None] * T))           # [B, NNEG*T]
    nl = nl.reshape(B, NNEG, T)
    idxf = np.zeros((B, T, CP), np.int16)
    idxf[:, :, 0] = np.arange(T, dtype=np.int16)[None, :]
    idxf[:, :, 1:COPIES] = nl.transpose(0, 2, 1)
    # copy CP-1 stays 0 (dummy)
    flat = idxf.reshape(B, NGT, NI)
    wrap = flat.reshape(B, NGT, NI // 16, 16).transpose(0, 1, 3, 2)  # [B,NGT,16,NI/16]
    idx_sb = np.ascontiguousarray(np.tile(wrap, (1, 1, 8, 1)))       # [B,NGT,128,NI/16]
    return x16, w16, yt16, idx_sb, nl


def kernel(x, y, W, b, neg_idxs, _trace=False):
    x = np.asarray(x, np.float32)
    y = np.asarray(y, np.float32)
    W = np.asarray(W, np.float32)
    b = np.asarray(b, np.float32)
    neg_idxs = np.asarray(neg_idxs)

    nc = _get_nc()
    x16, w16, yt16, idx_sb, nl = _prep_inputs(x, y, W, neg_idxs)

    in_maps = []
    for ci in range(NCORES):
        sl = slice(NB * ci, NB * (ci + 1))
        in_maps.append({
            "x2": np.ascontiguousarray(x16[sl]),
            "w": w16,
            "yt2": np.ascontiguousarray(yt16[sl]),
            "idx2": np.ascontiguousarray(idx_sb[sl]),
        })

    res = run_bass_kernel_spmd(nc, in_maps, core_ids=list(range(NCORES)),
                               trace=_trace)
    _CACHE["last_result"] = res

    # ---- host-side diagonal extraction + assembly ----
    P = np.empty((B, T, CP, S), np.float32)
    ar = np.arange(ST_U)
    for ci in range(NCORES):
        pr = res.results[ci]["praw"].astype(np.float32)  # [NB, PRAW_G, 96, 384]
        R = pr.reshape(NB, PRAW_G, ST_U, CP, NPS, S, ST_U)
        # diagonal over u_loc (axes 2 and 6) -> [ST_U, NB, PRAW_G, CP, NPS, S]
        D = R[:, :, ar, :, :, :, ar]
        # -> [NB, PRAW_G, NPS, ST_U, CP, S] -> [NB, T, CP, S]
        P[NB * ci:NB * (ci + 1)] = (
            D.transpose(1, 2, 4, 0, 3, 5).reshape(NB, T, CP, S))

    # exact bias correction: preds += dot(bias, target_column)
    if np.any(b):
        dby = np.einsum("d,bdt->bt", b, y)               # [B, T]
        corr = np.empty((B, T, COPIES), np.float32)
        corr[:, :, 0] = dby
        bidx = np.arange(B)[:, None, None]
        corr[:, :, 1:] = dby[bidx, nl.transpose(0, 2, 1)]
        P[:, :, :COPIES, :] += corr[:, :, :, None]

    preds = []
    for i in range(S):
        off = i + OFFSET
        blk = P[:, off:T, :COPIES, i]          # [B, T-off, COPIES]
        preds.append(np.ascontiguousarray(blk.transpose(1, 0, 2)).reshape(-1, COPIES))
    predictions = np.concatenate(preds, axis=0)
    labels = np.zeros((predictions.shape[0],), np.int32)
    return predictions, labels
